# revision 39
# baseline (speedup 1.0000x reference)
"""Multi-head attention (b=2, t=2048, E=1024, h=16) on 8 Trainium2 cores.

Sharding: tensor-parallel over heads - 2 heads per core. Each core computes
Q/K/V for its heads from the (replicated, pre-transposed) x, runs attention,
applies its slice of W_out, and emits a full-shape partial output. The host
sums the 8 partials.

v2: single flat pipeline - QKV projections and out-projections are spread as
filler work INSIDE the attention beat stream so the PE never idles (TRN2's
PE clock drops to ~1.2GHz after any idle gap and needs ~3us of continuous
execution to return to 2.4GHz). The exp train (ACT engine, ~1.1us per
[128,1024] tile) hides under the PE stream.

Device-side layout (kept from v1):
- Scores are computed TRANSPOSED (St[j, i], key index j on psum partitions),
  so softmax's sum-over-keys folds into the P@V matmul. Max-subtraction is
  skipped: |S/sqrt(d)| < 10 for this problem, exp() is safe.
- K^T for both heads stacked in one [128, NI] tensor; score matmuls contract
  their head's 64 partitions.
- V^T tiles are 256 cols: [V_A(64)|ones(64)|V_B(64)|ones(64)]; the ones half
  replicates the softmax denominator into psum partitions 64:127.

v2 changes:
- V is projected directly token-major on the PE (stationary = x slice,
  moving = Wv e-block), eliminating the separate transpose pass entirely.
- PSUM budget (16KB/partition exact): s-ring 2x[128,1024]f32 (S tiles),
  oA+oB [128,512]f32 (PV accum), proj [128,512]f32 (QKV), op [128,512]f32
  (out-projection halves).
- Lead-in projects chunks 0+1 (K,Q,V) paced by the x DMA stream; chunks 2..7
  project inside beat aux slots just-in-time; out-projection of chunk n runs
  during chunk n+1's beats.
- exp issues stay glued to their S matmuls (1-beat lookahead); all DMA
  issues during the beat stream live on the sync ring so the scalar (ACT)
  sequencer runs nothing but the exp train.
"""

import numpy as np
import ml_dtypes

import concourse.bass as bass
import concourse.mybir as mybir
import concourse.tile as tile
from concourse import bacc
from concourse.bass_utils import run_bass_kernel_spmd

F32 = mybir.dt.float32
BF16 = mybir.dt.bfloat16
AF = mybir.ActivationFunctionType

B = 2          # batch
T = 2048       # tokens per batch
E = 1024       # embed
H = 16         # heads
D = 64         # head dim
NC = 8         # cores
HPC = H // NC  # heads per core = 2
NI = B * T     # 4096 flattened tokens
DK = float(D) ** 0.5

EC = E // 128        # 8 contraction chunks for projections
NCH = NI // 512      # 8 global 512-token chunks
JT = T // 128        # 16 key tiles per batch
JP = JT // 2         # 8 jp-steps per chunk


def _build_nc():
    nc = bacc.Bacc("TRN2", target_bir_lowering=False, debug=False,
                   enable_asserts=False)

    xT = nc.dram_tensor("xT", [E, NI], BF16, kind="ExternalInput")
    wqT = nc.dram_tensor("wqT", [128, E], BF16, kind="ExternalInput")
    wkT = nc.dram_tensor("wkT", [128, E], BF16, kind="ExternalInput")
    wvT = nc.dram_tensor("wvT", [128, E], BF16, kind="ExternalInput")
    woT = nc.dram_tensor("woT", [128, E], BF16, kind="ExternalInput")
    out = nc.dram_tensor("out", [NI, E], BF16, kind="ExternalOutput")

    with tile.TileContext(nc) as tc:
        with (
            tc.tile_pool(name="persist", bufs=1) as persist,
            tc.tile_pool(name="xt", bufs=16) as xt_pool,
            tc.tile_pool(name="pt", bufs=3) as pt_pool,
            tc.tile_pool(name="norm", bufs=2) as norm_pool,
            tc.tile_pool(name="outc", bufs=4) as outc_pool,
            tc.tile_pool(name="ps", bufs=1, space="PSUM") as ps,
        ):
            # ---- persistent SBUF tensors ----
            wq_sb = persist.tile([128, E], BF16, name="wq_sb")
            wk_sb = persist.tile([128, E], BF16, name="wk_sb")
            wv_sb = persist.tile([128, E], BF16, name="wv_sb")
            wo_sb = persist.tile([128, E], BF16, name="wo_sb")
            qt_sb = persist.tile([128, NI], BF16, name="qt_sb")
            ktp = persist.tile([128, NI], BF16, name="ktp")
            va_sb = persist.tile([128, (JT * B) * 256], BF16, name="va_sb")
            ot_a = persist.tile([128, T], BF16, name="ot_a")
            ot_b = persist.tile([128, T], BF16, name="ot_b")
            ots = [ot_a, ot_b]

            # ---- input DMAs: x tiles alternate sync/vector rings so both
            # 8-queue groups stream; weights on the scalar ring (idle until
            # the exp train starts). Tiles (ip, e) = tokens ip*1024 +-, rows
            # e*128 +-. Fetch lead-in tiles (ip=0) first.
            xt_tiles = {}

            def fetch_x(ip, e, ring):
                t = xt_pool.tile([128, 1024], BF16, tag="xt", bufs=32)
                ring.dma_start(
                    t[:], xT[e * 128:(e + 1) * 128,
                             ip * 1024:(ip + 1) * 1024])
                xt_tiles[(ip, e)] = t

            # scalar ring: odd x tiles interleaved with e-ordered weight
            # pieces so the x-paced lead-in (which consumes w[:, e-slice] at
            # tile-e arrival) never waits on either stream. The first two
            # tiles fetch in column halves so the very first matmuls (which
            # read cols 0:512) start ~1.5us earlier.
            def fetch_x_split(ip, e, ring):
                t = xt_pool.tile([128, 1024], BF16, tag="xt", bufs=32,
                                 name="t_split")
                rows = slice(e * 128, (e + 1) * 128)
                ring.dma_start(t[:, 0:512],
                               xT[rows, ip * 1024:ip * 1024 + 512])
                ring.dma_start(t[:, 512:1024],
                               xT[rows, ip * 1024 + 512:(ip + 1) * 1024])
                xt_tiles[(ip, e)] = t

            # balance lead-in bytes across both rings (~1.5MB each):
            # sync carries even ip0 tiles + wk + wv, scalar carries odd
            # tiles + wq + wo, all e-ordered so the x-paced lead-in never
            # waits on a weight piece
            fetch_x_split(0, 0, nc.sync)
            fetch_x_split(0, 1, nc.scalar)
            nc.sync.dma_start(wk_sb[:, 0:128], wkT[:, 0:128])
            nc.scalar.dma_start(wq_sb[:, 0:256], wqT[:, 0:256])
            nc.sync.dma_start(wk_sb[:, 128:E], wkT[:, 128:E])
            fetch_x(0, 2, nc.sync)
            fetch_x(0, 3, nc.scalar)
            nc.scalar.dma_start(wq_sb[:, 256:E], wqT[:, 256:E])
            fetch_x(0, 4, nc.sync)
            fetch_x(0, 5, nc.scalar)
            nc.sync.dma_start(wv_sb[:], wvT[:, :])
            fetch_x(0, 6, nc.sync)
            fetch_x(0, 7, nc.scalar)
            for e in range(EC):
                fetch_x(1, e, nc.sync if e % 2 == 0 else nc.scalar)
            # wo isn't needed until the first out-projection (~beat 9)
            nc.scalar.dma_start(wo_sb[:], woT[:, :])
            for ip in range(2, 4):
                for e in range(EC):
                    fetch_x(ip, e, nc.sync)
            # V tile layout is [V_A(64)|ones(128)|V_B(64)]: head A's PV
            # stationary window (cols 0:128) is [V_A|ones], head B's (cols
            # 128:256) is [ones|V_B] - so B's psum lands den on partitions
            # 0:64 / V on 64:128, the mirror of A. This keeps the final
            # chunk's direct-from-psum normalization partition-aligned.
            va_ones = va_sb[:].rearrange(
                "p (t u) -> p t u", u=256)[:, :, 64:192]
            nc.gpsimd.memset(va_ones, 1.0)

            W_SB = {"q": wq_sb, "k": wk_sb, "v": wv_sb}

            # ---- projection helpers ----
            def proj_mm(kind, ci, e, pst, psl=slice(0, 512)):
                # one e-slice matmul of chunk ci's q/k projection (dim-major)
                ip, half = divmod(ci, 2)
                hsl = slice(half * 512, (half + 1) * 512)
                nc.tensor.matmul(
                    pst[:, psl], W_SB[kind][:, e * 128:(e + 1) * 128],
                    xt_tiles[(ip, e)][:, hsl],
                    start=(e == 0), stop=(e == EC - 1),
                    skip_group_check=True)

            def proj_copy(kind, ci, pst, psl=slice(0, 512)):
                isl = slice(ci * 512, (ci + 1) * 512)
                dst = ktp if kind == "k" else qt_sb
                with nc.allow_low_precision(reason="bf16 compute"):
                    nc.vector.tensor_copy(dst[:, isl], pst[:, psl])

            def projv_mm(tk, e, pst, c0):
                # V projected directly token-major: out[token, vdim] for one
                # 128-token tile, accumulated over e. Stationary = x slice,
                # moving = wv e-block rows (128 e x 128 vdims).
                ip, sl4 = divmod(tk, 8)
                nc.tensor.matmul(
                    pst[:, c0:c0 + 128],
                    xt_tiles[(ip, e)][:, sl4 * 128:(sl4 + 1) * 128],
                    wv_sb[:, e * 128:(e + 1) * 128],
                    start=(e == 0), stop=(e == EC - 1),
                    skip_group_check=True)

            def projv_copy(tk, pst, c0):
                # psum [token, 128 dims] -> va cols {0:64 (V_A),
                # 192:256 (V_B)}
                with nc.allow_low_precision(reason="bf16 compute"):
                    nc.vector.tensor_copy(
                        va_sb[:, tk * 256:tk * 256 + 64],
                        pst[:, c0:c0 + 64])
                    nc.vector.tensor_copy(
                        va_sb[:, tk * 256 + 192:tk * 256 + 256],
                        pst[:, c0 + 64:c0 + 128])

            # ---- PE p-state warmup: the clock needs ~3us of continuous
            # execution to reach 2.4GHz, and the first real matmul can only
            # start once its x/weight DMAs land (~12us). Dummy matmuls on
            # (uninitialized) qt_sb into a scratch psum tile keep the PE
            # executing from the moment the instruction stream opens, so
            # the lead-in runs at full clock. The scratch is never read.
            warm = ps.tile([128, 512], F32, tag="w", bufs=1, name="warm")
            for _ in range(18):
                nc.tensor.matmul(warm[:], qt_sb[:, 0:128], qt_sb[:, 512:1024],
                                 start=True, stop=True, skip_group_check=True)

            # ---- lead-in: project chunks 0 and 1 (K,Q,V^T), paced by the x
            # stream (4x512 + 8x128 matmul cols per arriving e-tile). Psum:
            # chunk0's Q|K in one s-ring tile, V^T tiles 0..7 in the other,
            # K1 in proj tag, Q1 in op tag. Copies ordered so the beat
            # stream's earliest deps (ktp0, qt0, va0/1) drain first.
            s_qk0 = ps.tile([128, 1024], F32, tag="s", bufs=2)
            s_v01 = ps.tile([128, 1024], F32, tag="s", bufs=2)
            w_k1 = ps.tile([128, 512], F32, tag="w", bufs=1)
            x_q1 = ps.tile([128, 512], F32, tag="x", bufs=1)
            warm2 = ps.tile([128, 512], F32, tag="oA", bufs=1, name="warm2")
            for e in range(EC):
                proj_mm("k", 0, e, s_qk0, slice(512, 1024))
                proj_mm("q", 0, e, s_qk0, slice(0, 512))
                proj_mm("k", 1, e, w_k1)
                proj_mm("q", 1, e, x_q1)
                if e >= 4:
                    # bridge the back-half x-arrival gap so the PE clock
                    # stays at max while (0,5)/(0,7) stream in
                    for _ in range(4):
                        nc.tensor.matmul(warm2[:], qt_sb[:, 0:128],
                                         qt_sb[:, 512:1024],
                                         start=True, stop=True,
                                         skip_group_check=True)
            proj_copy("k", 0, s_qk0, slice(512, 1024))
            proj_copy("q", 0, s_qk0, slice(0, 512))
            # V^T tiles sequentially: concurrent accumulation groups must
            # not share a psum bank (interleaving them corrupts results)
            for tk in range(8):
                for e in range(EC):
                    projv_mm(tk, e, s_v01, tk * 128)
                projv_copy(tk, s_v01, tk * 128)
            proj_copy("k", 1, w_k1)
            proj_copy("q", 1, x_q1)

            # ---- aux filler for the beat stream ----
            # ACT needs ~2.3us/beat for the exp train while the attention
            # matmuls alone give the PE only ~1.7us/beat, and the 2-deep S
            # psum ring means ACT can never run ahead: any PE-light beat
            # stalls the PE on exp semaphores. So projection and out-
            # projection work is spread EVENLY (~0.6us/beat) as demand-
            # driven micro-units with deadline forcing, instead of in big
            # front-loaded groups.
            pend_w = {}

            # proj track (tag "w"): chunks 2..7; K/Q groups = 4 units of 2
            # e-matmuls; V groups = 4 units of one token-tile (8 x 128-col
            # matmuls + copy). Groups hold the w psum tile across their
            # units; groups are strictly sequential on the tag.
            def emit_proj_unit(kind, ci, u):
                if u == 0:
                    pend_w["proj"] = ps.tile([128, 512], F32, tag="w",
                                             bufs=1, name="w_proj")
                pst = pend_w["proj"]
                if kind == "v":
                    tk = ci * 4 + u
                    for e in range(EC):
                        projv_mm(tk, e, pst, u * 128)
                    projv_copy(tk, pst, u * 128)
                else:
                    for e in range(2 * u, 2 * u + 2):
                        proj_mm(kind, ci, e, pst)
                    if u == 3:
                        proj_copy(kind, ci, pst)
                if u == 3:
                    pend_w.pop("proj")

            # (kind, ci, deadline): group's last unit must emit by beat D
            def _D(kind, ci):
                bb, cl = divmod(ci, 4)
                if kind == "k":
                    return 32 * bb + 2 * cl - 2
                if kind == "v":
                    return 32 * bb + 2 * cl - 1
                return 8 * ci - 2
            proj_seq = [("k", 2), ("v", 2), ("k", 3), ("v", 3),
                        ("q", 2), ("q", 3),
                        ("k", 4), ("q", 4), ("v", 4),
                        ("k", 5), ("v", 5),
                        ("k", 6), ("v", 6),
                        ("k", 7), ("v", 7),
                        ("q", 5), ("q", 6), ("q", 7)]
            # stagger unit deadlines so groups drain gradually rather than
            # bursting entirely at the group deadline
            proj_fifo = [(kind, ci, u, _D(kind, ci) - 3 + u)
                         for kind, ci in proj_seq for u in range(4)]

            # op track (tag "x"; also "w" once proj is exhausted): each
            # out-projection tile is 2 halves (512-wide matmul + cast) in
            # different beats; a tile's DMA fires with its second half.
            def emit_op_unit(ci, k, half, tag="x", drain=False):
                bb = ci // 4
                t0 = (ci % 4) * 512 + k * 128
                g0 = ci * 512 + k * 128
                ot2h = ots[bb]
                if half == 0:
                    ps_op = ps.tile([128, 512], F32, tag=tag, bufs=1,
                                    name="ps_op")
                    oc = outc_pool.tile([128, 1024], BF16, tag="oc")
                    pend_w[(ci, k)] = (ps_op, oc)
                else:
                    ps_op, oc = pend_w.pop((ci, k))
                esl = slice(half * 512, (half + 1) * 512)
                nc.tensor.matmul(
                    ps_op[:], ot2h[:, t0:t0 + 128], wo_sb[:, esl],
                    start=True, stop=True, skip_group_check=True)
                with nc.allow_low_precision(reason="bf16 out"):
                    if drain and half == 1:
                        nc.scalar.copy(oc[:, esl], ps_op[:])
                    else:
                        nc.vector.tensor_copy(oc[:, esl], ps_op[:])
                if half == 1:
                    nc.sync.dma_start(out[g0:g0 + 128, :], oc[:])

            op_fifo = []

            def pump_aux(t):
                # one op half per beat (same-tile halves land in adjacent
                # beats so the cast never blocks the next matmul), then
                # proj units to ~0.55us; deadline-forced units emit
                # regardless of budget
                cols = 0
                if op_fifo:
                    ci, k, half = op_fifo.pop(0)
                    emit_op_unit(ci, k, half)
                    cols += 512
                while proj_fifo:
                    kind, ci, u, dl = proj_fifo[0]
                    if dl > t and cols >= 1300:
                        break
                    proj_fifo.pop(0)
                    emit_proj_unit(kind, ci, u)
                    cols += 1024

            def queue_op(n):
                for k in range(4):
                    op_fifo.append((n, k, 0))
                    op_fifo.append((n, k, 1))

            # ---- attention beat helpers (layouts as v1) ----
            def emit_s_half(ci, jp, head):
                bb = ci // 4
                gisl = slice(ci * 512, (ci + 1) * 512)
                psl = slice(0, 64) if head == 0 else slice(64, 128)
                ps_s = ps.tile([128, 1024], F32, tag="s", bufs=2)
                for h in range(2):
                    j = 2 * jp + h
                    jsl = slice((bb * JT + j) * 128,
                                (bb * JT + j + 1) * 128)
                    hs = slice(h * 512, (h + 1) * 512)
                    nc.tensor.matmul(
                        ps_s[:, hs], ktp[psl, jsl], qt_sb[psl, gisl],
                        start=True, stop=True, skip_group_check=True)
                p = pt_pool.tile([128, 1024], BF16,
                                 tag="pA" if head == 0 else "pB")
                with nc.allow_low_precision(reason="bf16 probs"):
                    nc.scalar.activation(p[:], ps_s[:], AF.Exp,
                                         scale=1.0 / DK)
                return p

            def emit_pv_half(ci, jp, p, ps_o, head):
                bb = ci // 4
                for h in range(2):
                    j = 2 * jp + h
                    vb = (bb * JT + j) * 256 + head * 128
                    hs = slice(h * 512, (h + 1) * 512)
                    nc.tensor.matmul(
                        ps_o[:], va_sb[:, vb:vb + 128], p[:, hs],
                        start=(j == 0), stop=(j == JT - 1),
                        skip_group_check=True)

            def emit_norm_copies(ci, ps_oh, cp, rs, head):
                # drain the PV psum right after its last PV matmul so the
                # next chunk's PV reuses the bank asap. Head A's psum is
                # [V|den] on partitions, head B's is [den|V] (flipped va
                # layout); copies land V in cp and den in rs with head A on
                # partitions 0:64, head B on 64:128.
                if head == 0:
                    nc.vector.tensor_copy(cp[0:64, :], ps_oh[0:64, :])
                    nc.vector.tensor_copy(rs[0:64, :], ps_oh[64:128, :])
                else:
                    nc.vector.tensor_copy(cp[64:128, :], ps_oh[64:128, :])
                    nc.vector.tensor_copy(rs[64:128, :], ps_oh[0:64, :])

            def finish_norm(ci, cp, rs, csl=slice(0, 512)):
                rc = norm_pool.tile([128, 512], F32, tag="rc")
                nc.vector.reciprocal_approx_fast(rc[:, csl], rs[:, csl])
                ot2h = ots[ci // 4]
                icsl = slice((ci % 4) * 512 + csl.start,
                             (ci % 4) * 512 + csl.stop)
                with nc.allow_low_precision(reason="bf16 attn out"):
                    nc.vector.tensor_mul(ot2h[:, icsl], cp[:, csl],
                                         rc[:, csl])

            # ---- the beat stream ----
            steps = [(ci, jp) for ci in range(NCH) for jp in range(JP)]
            pA = emit_s_half(steps[0][0], steps[0][1], 0)
            pB = emit_s_half(steps[0][0], steps[0][1], 1)
            ps_oA = ps_oB = None
            cp = rs = None
            pending_norm = None
            for t, (ci, jp) in enumerate(steps):
                if pending_norm is not None:
                    finish_norm(*pending_norm)
                    pending_norm = None
                    queue_op(ci - 1)
                if jp == 0:
                    ps_oA = ps.tile([128, 512], F32, tag="oA", bufs=1)
                    ps_oB = ps.tile([128, 512], F32, tag="oB", bufs=1)
                nstep = steps[t + 1] if t + 1 < len(steps) else None
                nA = emit_s_half(nstep[0], nstep[1], 0) if nstep else None
                emit_pv_half(ci, jp, pA, ps_oA, 0)
                if jp == JP - 1 and ci < NCH - 1:
                    cp = norm_pool.tile([128, 512], F32, tag="cp")
                    rs = norm_pool.tile([128, 512], F32, tag="rs")
                    emit_norm_copies(ci, ps_oA, cp, rs, 0)
                nB = emit_s_half(nstep[0], nstep[1], 1) if nstep else None
                emit_pv_half(ci, jp, pB, ps_oB, 1)
                if jp == JP - 1 and ci < NCH - 1:
                    emit_norm_copies(ci, ps_oB, cp, rs, 1)
                    pending_norm = (ci, cp, rs)
                pA, pB = nA, nB
                pump_aux(t)

            # ---- tail: chunk 7 normalizes DIRECTLY from its PV psums
            # (flipped-B va layout makes both muls partition-aligned): per
            # 256-col half, 2 shifted den copies + recip + 2 psum-input
            # muls, interleaved with the out-projection. Casts alternate
            # ACT (idle after the last exp) and DVE; DMAs alternate rings.
            assert not proj_fifo
            for (n, k, half) in op_fifo:
                emit_op_unit(n, k, half, drain=True)
            rs7 = norm_pool.tile([128, 512], F32, tag="rs")
            rc7 = norm_pool.tile([128, 512], F32, tag="rc")
            for k in range(4):
                csl = slice(k * 128, (k + 1) * 128)
                t0, g0 = 1536 + k * 128, 3584 + k * 128
                osl = slice(t0, t0 + 128)
                with nc.allow_low_precision(reason="bf16 attn out"):
                    nc.vector.tensor_copy(rs7[0:64, csl],
                                          ps_oA[64:128, csl])
                    nc.vector.tensor_copy(rs7[64:128, csl],
                                          ps_oB[0:64, csl])
                    nc.vector.reciprocal_approx_fast(rc7[:, csl],
                                                     rs7[:, csl])
                    nc.vector.tensor_mul(ot_b[0:64, osl],
                                         ps_oA[0:64, csl], rc7[0:64, csl])
                    nc.vector.tensor_mul(ot_b[64:128, osl],
                                         ps_oB[64:128, csl],
                                         rc7[64:128, csl])
                ps_op = ps.tile([128, 1024], F32, tag="s", bufs=2)
                oc = outc_pool.tile([128, 1024], BF16, tag="oc")
                for half in range(2):
                    esl = slice(half * 512, (half + 1) * 512)
                    nc.tensor.matmul(
                        ps_op[:, esl], ot_b[:, t0:t0 + 128],
                        wo_sb[:, esl],
                        start=True, stop=True, skip_group_check=True)
                    with nc.allow_low_precision(reason="bf16 out"):
                        if (2 * k + half) % 2 == 0:
                            nc.scalar.copy(oc[:, esl], ps_op[:, esl])
                        else:
                            nc.vector.tensor_copy(oc[:, esl],
                                                  ps_op[:, esl])
                ring = nc.scalar if k % 2 == 0 else nc.sync
                ring.dma_start(out[g0:g0 + 128, :], oc[:])
    nc.compile()
    return nc


_CACHE = {}


def _get_nc():
    if "nc" not in _CACHE:
        _CACHE["nc"] = _build_nc()
    return _CACHE["nc"]


def _sb_layout(w):
    # [E, 128] -> SBUF layout [128, E]: sb[p, e*128+d] = w[e*128+p, d]
    return np.ascontiguousarray(
        w.reshape(EC, 128, 128).transpose(1, 0, 2).reshape(128, E))


def _prep_in_maps(x, W_qkv, W_out):
    bf16 = ml_dtypes.bfloat16
    xT = np.ascontiguousarray(x.reshape(NI, E).T).astype(bf16)
    dd = np.arange(D)
    in_maps = []
    for c in range(NC):
        heads = [c * HPC + k for k in range(HPC)]
        rq = np.concatenate([dd * 48 + 0 * 16 + hh for hh in heads])
        rk = np.concatenate([dd * 48 + 1 * 16 + hh for hh in heads])
        rv = np.concatenate([dd * 48 + 2 * 16 + hh for hh in heads])
        cols = slice(c * 128, (c + 1) * 128)
        in_maps.append({
            "xT": xT,
            "wqT": _sb_layout(W_qkv[rq].T).astype(bf16),
            "wkT": _sb_layout(W_qkv[rk].T).astype(bf16),
            "wvT": _sb_layout(W_qkv[rv].T).astype(bf16),
            "woT": np.ascontiguousarray(W_out[:, cols].T).astype(bf16),
        })
    return in_maps


def run(x, W_qkv, W_out, trace=False, **spmd_kwargs):
    x = np.asarray(x, dtype=np.float32)
    W_qkv = np.asarray(W_qkv, dtype=np.float32)
    W_out = np.asarray(W_out, dtype=np.float32)
    nc = _get_nc()
    in_maps = _prep_in_maps(x, W_qkv, W_out)
    res = run_bass_kernel_spmd(nc, in_maps, core_ids=list(range(NC)),
                               trace=trace, **spmd_kwargs)
    acc = res.results[0]["out"].astype(np.float32)
    for c in range(1, NC):
        acc = acc + res.results[c]["out"]
    return acc.reshape(B, T, E), res


def kernel(x, W_qkv, W_out):
    out, _ = run(x, W_qkv, W_out)
    return out
